# revision 6
# baseline (speedup 1.0000x reference)
"""Trainium2 Bass kernel for nn_CompressedInteractionNet_31997506355236.

Reference math (per batch b, channel k, dim d; m == H == 64, D == 16, vk == 16):
    x0r[b,d,:]  = x_0[b,:,d]                      # [m]
    xhr[b,d,:]  = x_0[b].reshape(D, H)[d]         # [H] (flat reinterpretation)
    out[b,k,d]  = sum_v (x0r[b,d] @ Vm[k,0,:,v]) * (Vh[k,0,v,:] @ xhr[b,d])

Sharding: batch x channels = 4 x 2 over 8 cores (32 batches, 32 channels per
core). Inputs are cast to bf16 host-side (tolerance 2e-2; bf16 keeps matmul
at 1 cycle/row and halves DMA bytes).

Device schedule per 128-row chunk c (4 chunks):
    Bt = xhrt_c.T @ vhf   (PE -> PSUM)      A = x0t_c.T @ vmf  (PE -> PSUM)
    b_sb = copy(Bt)       (ACT; DVE allows <=1 PSUM input)
    P = A * b_sb          (DVE)
    o = sum_v P           (Pool folds + DVE finish; last chunk direct DVE
                           reduce for the shortest tail chain)
    dma out               (SP)
All four input DMAs ride the two HW-DGE queues (SP and ACT, two DMAs each;
the gpsimd SW-DGE queue issues ~0.7us later so it gets nothing). B-side
operands (xhrt, vhf) go first on each queue so the Bt->copy chain starts
as early as possible.
"""

import numpy as np
import ml_dtypes

import concourse.bass as bass
import concourse.tile as tile
from concourse import bacc, mybir
from concourse.bass_utils import run_bass_kernel_spmd

# Problem constants (hardcoded; kernel must be self-contained).
B, M, D = 128, 64, 16
HK, VK = 64, 16
H = 64
NCORES = 8
SB, SK = 4, 2             # batch shards x channel shards
BL = B // SB              # batches per core = 32
BD = BL * D               # rows per core = 512
KL = HK // SK             # channels per core = 32
KVL = KL * VK             # 512
NCH = BD // 128           # 128-row chunks per core = 4
F32 = mybir.dt.float32
BF16 = mybir.dt.bfloat16
BF = ml_dtypes.bfloat16

_CACHE = {}


def build_bass():
    nc = bacc.Bacc("TRN2", target_bir_lowering=False, debug=False,
                   num_devices=NCORES, enable_partition_id=False,
                   monotonic_sem_count=0)

    vhf_d = nc.dram_tensor("vhf", [H, KVL], BF16, kind="ExternalInput")
    xhrt_d = nc.dram_tensor("xhrt", [H, BD], BF16, kind="ExternalInput")
    x0t_d = nc.dram_tensor("x0t", [M, BD], BF16, kind="ExternalInput")
    vmf_d = nc.dram_tensor("vmf", [M, KVL], BF16, kind="ExternalInput")
    out = nc.dram_tensor("out", [BD, KL], F32, kind="ExternalOutput")

    with tile.TileContext(nc) as tc:
        with (
            tc.tile_pool(name="w", bufs=1) as w,
            tc.tile_pool(name="work", bufs=4) as work,
            tc.tile_pool(name="pa", bufs=4, space="PSUM") as pa,
            tc.tile_pool(name="pb", bufs=2, space="PSUM") as pb,
        ):
            # ---- input DMAs: B-side first on each HW-DGE queue ---------
            vhf = w.tile([H, KVL], BF16)
            nc.sync.dma_start(vhf[:], vhf_d.ap())
            xhrt = w.tile([H, BD], BF16)
            nc.scalar.dma_start(xhrt[:], xhrt_d.ap())
            x0t = w.tile([M, BD], BF16)
            nc.sync.dma_start(x0t[:], x0t_d.ap())
            vmf = w.tile([M, KVL], BF16)
            nc.scalar.dma_start(vmf[:], vmf_d.ap())

            # Phase-ordered emission: the PE/ACT/DVE mul pipeline first so
            # the DVE never wedges a reduce between muls; Pool folds and the
            # final reduces trail. c0/c1 fold deep on Pool (it has slack),
            # c2 shallow, c3 reduces directly on DVE (shortest tail chain).
            psum_as, psum_bs, p_sbs = [], [], []
            for c in range(NCH):
                off = 128 * c
                psum_b = pb.tile([128, KVL], F32, tag="b")
                nc.tensor.matmul(psum_b[:], xhrt[:, off:off + 128], vhf[:],
                                 start=True, stop=True)
                psum_a = pa.tile([128, KVL], F32, tag="a")
                nc.tensor.matmul(psum_a[:], x0t[:, off:off + 128], vmf[:],
                                 start=True, stop=True)

                b_sb = work.tile([128, KL, VK], F32, tag="b_sb")
                nc.scalar.copy(b_sb.rearrange("p k v -> p (k v)"), psum_b[:])
                p_sb = work.tile([128, KL, VK], F32, tag="p_sb")
                nc.vector.tensor_mul(
                    out=p_sb.rearrange("p k v -> p (k v)"),
                    in0=psum_a[:],
                    in1=b_sb.rearrange("p k v -> p (k v)"))
                p_sbs.append(p_sb)

            red_ins = {}
            for c in range(NCH - 1):
                t1 = work.tile([128, KL, VK // 2], F32, tag="t1")
                nc.gpsimd.tensor_tensor(t1[:], p_sbs[c][:, :, 0:8],
                                        p_sbs[c][:, :, 8:16],
                                        mybir.AluOpType.add)
                red_ins[c] = t1
                if c < 2:
                    t2 = work.tile([128, KL, VK // 4], F32, tag="t2")
                    nc.gpsimd.tensor_tensor(t2[:], t1[:, :, 0:4],
                                            t1[:, :, 4:8],
                                            mybir.AluOpType.add)
                    red_ins[c] = t2
            red_ins[NCH - 1] = p_sbs[NCH - 1]

            for c in (3, 0, 1, 2):
                o_sb = work.tile([128, KL], F32, tag="o_sb")
                nc.vector.tensor_reduce(out=o_sb[:], in_=red_ins[c][:],
                                        axis=mybir.AxisListType.X,
                                        op=mybir.AluOpType.add)
                nc.sync.dma_start(out.ap()[128 * c:128 * (c + 1), :], o_sb[:])

    nc.compile()
    return nc


def run(x_0, x_h, Vm, Vh, **spmd_kwargs):
    x_0 = np.ascontiguousarray(np.asarray(x_0), dtype=np.float32)
    vm = np.asarray(Vm)[:, 0].astype(np.float32)
    vh = np.asarray(Vh)[:, 0].astype(np.float32)

    # Host-side layout prep (part of sharding): all-contiguous bf16 inputs.
    vmf = vm.transpose(1, 0, 2).reshape(M, HK * VK).astype(BF)
    vhf = vh.transpose(2, 0, 1).reshape(H, HK * VK).astype(BF)

    if "nc" not in _CACHE:
        _CACHE["nc"] = build_bass()
    nc = _CACHE["nc"]

    in_maps = []
    for core in range(NCORES):
        cb, ck = divmod(core, SK)
        shard = x_0[BL * cb:BL * (cb + 1)]                    # [BL, M, D]
        x0t = shard.transpose(1, 0, 2).reshape(M, BD).astype(BF)
        xhrt = shard.reshape(BL, D, H).transpose(2, 0, 1).reshape(H, BD) \
            .astype(BF)
        ks = slice(KVL * ck, KVL * (ck + 1))
        in_maps.append({
            "vhf": np.ascontiguousarray(vhf[:, ks]),
            "xhrt": np.ascontiguousarray(xhrt),
            "x0t": np.ascontiguousarray(x0t),
            "vmf": np.ascontiguousarray(vmf[:, ks]),
        })

    res = run_bass_kernel_spmd(nc, in_maps, core_ids=list(range(NCORES)),
                               **spmd_kwargs)
    # Unshard: per-core out is [(b,d), k_loc] -> [BL, D, KL] -> [BL, KL, D]
    full = np.empty((B, HK, D), dtype=np.float32)
    for core in range(NCORES):
        cb, ck = divmod(core, SK)
        o = res.results[core]["out"].reshape(BL, D, KL).transpose(0, 2, 1)
        full[BL * cb:BL * (cb + 1), KL * ck:KL * (ck + 1), :] = o
    return full, res


def kernel(x_0, x_h, Vm, Vh):
    return run(x_0, x_h, Vm, Vh)[0]


if __name__ == "__main__":
    rng = np.random.default_rng(0)
    x_0 = rng.standard_normal((B, M, D)).astype(np.float32)
    x_h = rng.standard_normal((B, H, D)).astype(np.float32)
    Vm = rng.standard_normal((HK, 1, M, VK)).astype(np.float32)
    Vh = rng.standard_normal((HK, 1, VK, H)).astype(np.float32)
    got = kernel(x_0, x_h, Vm, Vh)

    x0r = np.transpose(x_0, (0, 2, 1))
    xhr = x_0.reshape(B, D, H)
    a = np.einsum("bdi,kiv->bkdv", x0r, Vm[:, 0])
    bb = np.einsum("bdj,kvj->bkdv", xhr, Vh[:, 0])
    want = np.einsum("bkdv,bkdv->bkd", a, bb)
    err = np.abs(got - want).max() / np.abs(want).max()
    print("rel err:", err)
